# revision 25
# baseline (speedup 1.0000x reference)
"""Causal self-attention (GQA, head-indexed RoPE) on 8 Trainium2 NeuronCores.

Sharding: 2-way data parallel over batch x 4-way tensor parallel over GQA
groups (4 q heads + 1 kv head per core). RoPE in this module is indexed by
the *head* axis (not sequence), so it is a fixed per-head linear map on the
head dim -- folded into the q/k projection weights on the host, along with
the 1/sqrt(d) scale (into q) and the v bias (into the c_proj bias).

All matmuls run in bf16 (same 1 row/cycle PE stream as fp32r, but weight
loads hit the fast-weight-load path and every byte count halves; measured
end-to-end rel err ~4e-3 vs the 2e-2 gate).

Device program per core (identical SPMD program, inputs differ):
  1. qT/kT/vT = W.T-slices @ xT (PSUM accum over C chunks, s-major so PSUM
     banks rotate without stalling the PE), +bias via ACT, v transposed to
     [t, d] layout with PE transposes.
  2. per head, per 512-wide q chunk, per 128-wide k block: S^T tile = K Q^T,
     exp on ACT, causal handled by restricting the diagonal blocks' column
     range (plus one 128x128 triangle multiply on DVE); P V and the ones-
     matmul row sums accumulate in PSUM with a 2-block software pipeline so
     the PE never waits on the ACT exp. DVE reciprocal + multiply normalize
     into yT. After each head, a 4-rank (batch-subgroup) AllToAll reshards
     that head's yT from head-sharded to t-sharded.
  3. c_proj is computed incrementally: as each head-shard lands, its 4
     contraction chunks are matmul'd (PSUM, rotating through the attention
     st pool's banks) and DVE-accumulated into SBUF f32 accumulators that
     were initialized with the output bias. Only the last head-shard's
     round remains after attention ends.
"""

import numpy as np
import ml_dtypes

import concourse.mybir as mybir
import concourse.tile as tile
from concourse import bacc
from concourse.bass_utils import run_bass_kernel_spmd

F32 = mybir.dt.float32
BF16 = mybir.dt.bfloat16
NP_BF16 = ml_dtypes.bfloat16

# Model constants (hardcoded per contract)
B, T_FULL, C = 2, 2048, 2048
NH, G = 16, 4
NKV = NH // G          # 4 kv heads = 4 groups
HD = C // NH           # 128
BASE = 10000.0
N_CORES = 8
HPC = NH // NKV        # 4 q heads per core


# --------------------------------------------------------------------------
# device program
# --------------------------------------------------------------------------

def build_nc(T=T_FULL):
    assert T % 512 == 0
    TC = T // 512          # 512-wide q chunks in attention
    KB = T // 128          # 128-wide k blocks
    CCH = C // 128         # contraction chunks over C
    TS = T // NKV          # c_proj t-shard rows per core
    NTL = TS // 128        # 128-row t tiles in this core's shard

    nc = bacc.Bacc(
        "TRN2",
        target_bir_lowering=False,
        debug=False,
        enable_asserts=False,
        num_devices=N_CORES,
    )

    xT_d = nc.dram_tensor("xT", [128, TC, CCH, 512], BF16, kind="ExternalInput").ap()
    wT_d = nc.dram_tensor("wT", [128, CCH, 6 * HD], BF16, kind="ExternalInput").ap()
    qkb_d = nc.dram_tensor("qkb", [HD, 5], F32, kind="ExternalInput").ap()
    tri_d = nc.dram_tensor("tri", [128, 128], BF16, kind="ExternalInput").ap()
    ones_d = nc.dram_tensor("ones", [128, 128], BF16, kind="ExternalInput").ap()
    ident_d = nc.dram_tensor("ident", [128, 128], BF16, kind="ExternalInput").ap()
    sel_d = nc.dram_tensor("sel", [128, 2], F32, kind="ExternalInput").ap()
    cpT_d = nc.dram_tensor("cpT", [128, HPC, NKV, C], BF16, kind="ExternalInput").ap()
    cpb_d = nc.dram_tensor("cpb", [128, C], F32, kind="ExternalInput").ap()
    out_d = nc.dram_tensor("out", [TS, C], F32, kind="ExternalOutput").ap()

    Act = mybir.ActivationFunctionType
    Alu = mybir.AluOpType
    RG = [list(range(N_CORES))]

    with tile.TileContext(nc) as tc:
        # ---------- DRAM singles for the per-head AllToAll (content
        # duplicated to both batch groups to stay SPMD-symmetric; the
        # receiver selects its batch half with a per-core 0/1 mask) ----------
        warm_in, free_wi = tc.tile([N_CORES * 128, 4], BF16, space="DRAM",
                                   name="a2a_warm_in")
        warm_out, free_wo = tc.tile([N_CORES * 128, 4], BF16, space="DRAM",
                                    name="a2a_warm_out")
        a2a_in, a2a_out, a2a_free = [], [], []
        a2a_free.append(free_wi)
        a2a_free.append(free_wo)
        for h in range(HPC):
            ti, fi = tc.tile([N_CORES * 128, TS], BF16, space="DRAM",
                             name=f"a2a_in{h}")
            a2a_in.append(ti)
            a2a_free.append(fi)
        for h in range(HPC):
            to, fo = tc.tile([N_CORES * 128, TS], BF16, space="DRAM",
                             name=f"a2a_out{h}")
            a2a_out.append(to)
            a2a_free.append(fo)


        # ---------- persistent SBUF ----------
        frees = []

        def ptile(shape, dt, name):
            t, f = tc.tile(shape, dt, name=name)
            frees.append(f)
            return t

        out_acc = ptile([128, 16, 512], F32, "out_acc")
        cpb_sb = ptile([128, C], F32, "cpb_sb")
        qk_sb = ptile([128, 5, T], BF16, "qk_sb")     # 4 qT + kT, d-major
        v_sb = ptile([128, KB, 128], BF16, "v_sb")    # v in [t, d] layout
        yT_sb = ptile([128, HPC, T], BF16, "yT_sb")
        ybig = [ptile([128, CCH, 128], BF16, f"ybig{tl}") for tl in range(NTL)]
        ones_sb = ptile([128, 128], BF16, "ones_sb")
        ident_sb = ptile([128, 128], BF16, "ident_sb")
        tri_sb = ptile([128, 128], BF16, "tri_sb")
        qkb_sb = ptile([128, 5], F32, "qkb_sb")
        sel_sb = ptile([128, 2], F32, "sel_sb")

        nc.sync.dma_start(sel_sb[:], sel_d)
        nc.sync.dma_start(cpb_sb[:], cpb_d)
        nc.sync.dma_start(ones_sb[:], ones_d)
        nc.sync.dma_start(ident_sb[:], ident_d)
        nc.sync.dma_start(tri_sb[:], tri_d)
        nc.sync.dma_start(qkb_sb[:], qkb_d)

        # tiny throwaway AllToAll: absorbs NRT's first-collective trigger
        # latency + ring warmup during phase 1 (payload content irrelevant,
        # but it needs a producer so its completion increment posts and the
        # later collectives' waits aren't all shifted one op late)
        nc.sync.dma_start(warm_in[0:128, :], ones_sb[:, 0:4])
        nc.gpsimd.collective_compute(
            "AllToAll",
            mybir.AluOpType.bypass,
            replica_groups=[list(range(N_CORES))],
            ins=[warm_in.opt()],
            outs=[warm_out.opt()],
        )

        # ---------- phase 1: qkv projections ----------
        with (
            tc.tile_pool(name="w_pool", bufs=1) as w_pool,
            tc.tile_pool(name="x_pool", bufs=3) as x_pool,
            tc.tile_pool(name="qkv_ps", bufs=1, space="PSUM") as qkv_ps,
            tc.tile_pool(name="tp_ps", bufs=2, space="PSUM") as tp_ps,
            tc.tile_pool(name="vtmp_pool", bufs=2) as vtmp_pool,
        ):
            # per-cc-slice DMAs so the first matmuls only wait on their own
            # 1KB/partition slice, not the whole 3MB/2MB transfer
            w_sb = w_pool.tile([128, CCH, 6 * HD], BF16, name="w_sb")

            def drain_s(s, t, ps):
                if s < 5:
                    nc.scalar.activation(
                        qk_sb[:, s, t * 512:(t + 1) * 512],
                        ps[s][:],
                        Act.Identity,
                        bias=qkb_sb[:, s:s + 1],
                        scale=1.0,
                    )
                else:
                    vtmp = vtmp_pool.tile([128, 512], BF16, name="vtmp")
                    nc.any.tensor_copy(out=vtmp[:], in_=ps[5][:])
                    for i in range(4):
                        tp = tp_ps.tile([128, 128], BF16, name="tp")
                        nc.tensor.transpose(
                            tp[:], vtmp[:, i * 128:(i + 1) * 128], ident_sb[:]
                        )
                        nc.any.tensor_copy(out=v_sb[:, t * 4 + i, :], in_=tp[:])

            for t in range(TC):
                xt = x_pool.tile([128, CCH, 512], BF16, name="xt")
                for cc in range(CCH):
                    if t == 0:
                        nc.sync.dma_start(w_sb[:, cc, :], wT_d[:, cc, :])
                    nc.sync.dma_start(xt[:, cc, :], xT_d[:, t, cc, :])
                ps = [qkv_ps.tile([128, 512], F32, name=f"qkv{s}") for s in range(6)]
                if t == 0:
                    # cc-major: consumes one (w, x) slice pair per 6 matmuls,
                    # matching the DMA arrival rate at kernel start
                    for cc in range(CCH):
                        for s in range(6):
                            nc.tensor.matmul(
                                ps[s][:],
                                lhsT=w_sb[:, cc, s * 128:(s + 1) * 128],
                                rhs=xt[:, cc, :],
                                start=(cc == 0),
                                stop=(cc == CCH - 1),
                            )
                    for s in range(6):
                        drain_s(s, t, ps)
                else:
                    # s-major: each PSUM bank finishes early and drains while
                    # the next one computes
                    for s in range(6):
                        for cc in range(CCH):
                            nc.tensor.matmul(
                                ps[s][:],
                                lhsT=w_sb[:, cc, s * 128:(s + 1) * 128],
                                rhs=xt[:, cc, :],
                                start=(cc == 0),
                                stop=(cc == CCH - 1),
                            )
                        drain_s(s, t, ps)

        # ---------- phase 2: attention + per-head AllToAll + streamed c_proj
        with (
            tc.tile_pool(name="st_ps", bufs=3, space="PSUM") as st_ps,
            tc.tile_pool(name="yr_ps", bufs=2, space="PSUM") as yr_ps,
            tc.tile_pool(name="pt_pool", bufs=4) as pt_pool,
            tc.tile_pool(name="nrm_pool", bufs=2) as nrm_pool,
            tc.tile_pool(name="cp_pool", bufs=4) as cp_pool,
            tc.tile_pool(name="ysel_pool", bufs=4) as ysel_pool,
        ):
            cpT_tiles = {}

            def ysel(hs):
                # select this core's batch half of head-shard hs into ybig
                for tl in range(NTL):
                    lo = ysel_pool.tile([128, NKV, 128], BF16, name="lo")
                    hi = ysel_pool.tile([128, NKV, 128], BF16, name="hi")
                    nc.sync.dma_start(
                        lo[:],
                        a2a_out[hs][0:512, tl * 128:(tl + 1) * 128]
                        .rearrange("(g p) t -> p g t", p=128),
                    )
                    nc.sync.dma_start(
                        hi[:],
                        a2a_out[hs][512:1024, tl * 128:(tl + 1) * 128]
                        .rearrange("(g p) t -> p g t", p=128),
                    )
                    tmp = ysel_pool.tile([128, NKV, 128], BF16, name="tmp")
                    nc.vector.tensor_scalar_mul(tmp[:], lo[:], sel_sb[:, 0:1])
                    nc.vector.scalar_tensor_tensor(
                        out=ybig[tl][:, NKV * hs:NKV * (hs + 1), :],
                        in0=hi[:],
                        scalar=sel_sb[:, 1:2],
                        in1=tmp[:],
                        op0=Alu.mult,
                        op1=Alu.add,
                    )

            def fetch_cpT(hs):
                ct = cp_pool.tile([128, NKV, C], BF16, name="cpT")
                nc.sync.dma_start(ct[:], cpT_d[:, hs])
                cpT_tiles[hs] = ct

            def cproj_round(hs):
                # accumulate head-shard hs's 4 contraction chunks into the
                # SBUF f32 accumulators (init with bias on the first round)
                ct = cpT_tiles.pop(hs)
                for tl in range(NTL):
                    for j in range(4):
                        cps = st_ps.tile([128, 512], F32, name="stp")
                        for g in range(NKV):
                            nc.tensor.matmul(
                                cps[:],
                                lhsT=ybig[tl][:, NKV * hs + g, :],
                                rhs=ct[:, g, j * 512:(j + 1) * 512],
                                start=(g == 0),
                                stop=(g == NKV - 1),
                            )
                        idx = 4 * j + tl
                        prev = (cpb_sb[:, j * 512:(j + 1) * 512] if hs == 0
                                else out_acc[:, idx, :])
                        nc.vector.scalar_tensor_tensor(
                            out=out_acc[:, idx, :],
                            in0=cps[:],
                            scalar=1.0,
                            in1=prev,
                            op0=Alu.mult,
                            op1=Alu.add,
                        )
                        if hs == HPC - 1:
                            nc.sync.dma_start(
                                out_d[tl * 128:(tl + 1) * 128,
                                      j * 512:(j + 1) * 512],
                                out_acc[:, idx, :],
                            )

            for hs in range(HPC):
                fetch_cpT(hs)

            for h in range(HPC):
                for qc in range(TC):
                    kb_max = 4 * qc + 4
                    yps = yr_ps.tile([128, 512], F32, name="yps")
                    rps = yr_ps.tile([128, 512], F32, name="rps")
                    pend = []

                    def flush_one():
                        kb0, pt0, c00 = pend.pop(0)
                        nc.tensor.matmul(
                            yps[:, c00:512],
                            lhsT=v_sb[:, kb0, :],
                            rhs=pt0[:, c00:512],
                            start=(kb0 == 0),
                            stop=(kb0 == kb_max - 1),
                        )
                        nc.tensor.matmul(
                            rps[:, c00:512],
                            lhsT=ones_sb[:],
                            rhs=pt0[:, c00:512],
                            start=(kb0 == 0),
                            stop=(kb0 == kb_max - 1),
                        )

                    for kb in range(kb_max):
                        m = kb - 4 * qc
                        c0 = 128 * m if m >= 0 else 0
                        stp = st_ps.tile([128, 512], F32, name="stp")
                        nc.tensor.matmul(
                            stp[:, c0:512],
                            lhsT=qk_sb[:, 4, kb * 128:(kb + 1) * 128],
                            rhs=qk_sb[:, h, qc * 512 + c0:(qc + 1) * 512],
                            start=True,
                            stop=True,
                        )
                        pt = pt_pool.tile([128, 512], BF16, name="pt")
                        nc.scalar.activation(pt[:, c0:512], stp[:, c0:512], Act.Exp)
                        if m >= 0:
                            nc.vector.tensor_mul(
                                out=pt[:, c0:c0 + 128],
                                in0=pt[:, c0:c0 + 128],
                                in1=tri_sb[:],
                            )
                        pend.append((kb, pt, c0))
                        if len(pend) > 2:
                            flush_one()
                    while pend:
                        flush_one()

                    # softmax normalize on DVE (ACT Ln+Exp would be cheaper
                    # per element but walrus puts Ln and Exp in different
                    # table sets -> 2.7us ACT_TABLE_LOAD per alternation)
                    recip = nrm_pool.tile([128, 512], F32, name="recip")
                    nc.vector.reciprocal(recip[:], rps[:])
                    nc.vector.tensor_mul(
                        out=yT_sb[:, h, qc * 512:(qc + 1) * 512],
                        in0=yps[:],
                        in1=recip[:],
                    )
                    # this qc's columns are exactly a2a chunk gd == qc:
                    # ship them now (identical content to both batch groups'
                    # rank so the program stays core-independent)
                    for bd in range(2):
                        j = 4 * bd + qc
                        nc.sync.dma_start(
                            a2a_in[h][128 * j:128 * (j + 1), :],
                            yT_sb[:, h, qc * TS:(qc + 1) * TS],
                        )
                    # head h-2's select runs mid-head on otherwise-idle DVE
                    # slots; its AllToAll landed a full head ago
                    if qc == 1 and h >= 2:
                        ysel(h - 2)

                nc.gpsimd.collective_compute(
                    "AllToAll",
                    Alu.bypass,
                    replica_groups=RG,
                    ins=[a2a_in[h].opt()],
                    outs=[a2a_out[h].opt()],
                )
                if h >= 2:
                    cproj_round(h - 2)

            ysel(HPC - 2)
            cproj_round(HPC - 2)
            ysel(HPC - 1)
            cproj_round(HPC - 1)

        for f in reversed(frees):
            f()
        for f in reversed(a2a_free):
            f()

    nc.compile()
    return nc


# --------------------------------------------------------------------------
# host-side preparation
# --------------------------------------------------------------------------

def _rot_mat(angle_idx):
    half = HD // 2
    theta = 1.0 / (BASE ** (np.arange(half, dtype=np.float64) * 2.0 / HD))
    ang = angle_idx * theta
    c, s = np.cos(ang), np.sin(ang)
    R = np.zeros((HD, HD), dtype=np.float64)
    idx = np.arange(half)
    R[2 * idx, 2 * idx] = c
    R[2 * idx, 2 * idx + 1] = -s
    R[2 * idx + 1, 2 * idx] = s
    R[2 * idx + 1, 2 * idx + 1] = c
    return R


def prep_in_maps(inputs, T=T_FULL):
    TC = T // 512
    CCH = C // 128
    x = np.asarray(inputs["x"], np.float32)
    wq_w = np.asarray(inputs["wq_w"], np.float64)
    wq_b = np.asarray(inputs["wq_b"], np.float64)
    wk_w = np.asarray(inputs["wk_w"], np.float64)
    wk_b = np.asarray(inputs["wk_b"], np.float64)
    wv_w = np.asarray(inputs["wv_w"], np.float64)
    wv_b = np.asarray(inputs["wv_b"], np.float64)
    cp_w = np.asarray(inputs["cp_w"], np.float64)
    cp_b = np.asarray(inputs["cp_b"], np.float64)

    scale = 1.0 / np.sqrt(HD)
    wq_eff = np.empty((C, C), np.float64)
    bq_eff = np.empty(C, np.float64)
    for h in range(NH):
        R = _rot_mat(h)
        sl = slice(HD * h, HD * (h + 1))
        wq_eff[sl] = R @ wq_w[sl] * scale
        bq_eff[sl] = R @ wq_b[sl] * scale
    wk_eff = np.empty((C // G, C), np.float64)
    bk_eff = np.empty(C // G, np.float64)
    for g in range(NKV):
        R = _rot_mat(g)
        sl = slice(HD * g, HD * (g + 1))
        wk_eff[sl] = R @ wk_w[sl]
        bk_eff[sl] = R @ wk_b[sl]

    bv_exp = np.zeros(C, np.float64)
    for h in range(NH):
        g = h // G
        bv_exp[HD * h:HD * (h + 1)] = wv_b[HD * g:HD * (g + 1)]
    cp_b_eff = (cp_b + cp_w @ bv_exp).astype(np.float32)

    i = np.arange(128)
    tri = (i[None, :] >= i[:, None]).astype(NP_BF16)       # [kpos, qpos]
    ones = np.ones((128, 128), NP_BF16)
    ident = np.eye(128, dtype=NP_BF16)

    # c_proj weights packed [ci, hs, g, c] so the per-head-shard tile is a
    # contiguous DMA: row (hs, g, ci) <- cp column for global head (4g + hs)
    cpT_pack = np.empty((128, HPC, NKV, C), NP_BF16)
    for hs in range(HPC):
        for g in range(NKV):
            H = NKV * g + hs
            cpT_pack[:, hs, g, :] = \
                cp_w[:, 128 * H:128 * (H + 1)].astype(np.float32).T.astype(NP_BF16)
    cpb_full = np.tile(cp_b_eff[None, :], (128, 1))

    # x packed [ci, t_chunk, co, f] so each t-chunk is one contiguous DMA
    xT = []
    for b in range(B):
        arr = np.ascontiguousarray(x[b, :T].T).astype(NP_BF16)   # [C, T]
        xT.append(np.ascontiguousarray(
            arr.reshape(CCH, 128, TC, 512).transpose(1, 2, 0, 3)))

    in_maps = []
    for core in range(N_CORES):
        b, g = divmod(core, NKV)
        wq_g = wq_eff[512 * g:512 * (g + 1)]          # [512, C]
        wk_g = wk_eff[128 * g:128 * (g + 1)]          # [128, C]
        wv_g = wv_w[128 * g:128 * (g + 1)]            # [128, C]
        wT = np.concatenate(
            [wq_g.T, wk_g.T, wv_g.T], axis=1).astype(np.float32)  # [C, 768]
        wT_pack = np.ascontiguousarray(
            wT.reshape(CCH, 128, 6 * HD).transpose(1, 0, 2)).astype(NP_BF16)
        qkb = np.concatenate(
            [bq_eff[512 * g:512 * (g + 1)].astype(np.float32).reshape(4, 128).T,
             bk_eff[128 * g:128 * (g + 1)].astype(np.float32).reshape(1, 128).T],
            axis=1,
        )                                              # [128, 5]
        sel = np.zeros((128, 2), np.float32)
        sel[:, b] = 1.0
        in_maps.append({
            "xT": xT[b],
            "wT": wT_pack,
            "qkb": np.ascontiguousarray(qkb),
            "tri": tri,
            "ones": ones,
            "ident": ident,
            "sel": sel,
            "cpT": cpT_pack,
            "cpb": cpb_full,
        })
    return in_maps


def assemble(results, T=T_FULL):
    TS = T // NKV
    out = np.empty((B, T, C), np.float32)
    for core in range(N_CORES):
        b, g = divmod(core, NKV)
        out[b, TS * g:TS * (g + 1), :] = results[core]["out"]
    return out


_NC_CACHE = {}


def _get_nc(T=T_FULL):
    if T not in _NC_CACHE:
        _NC_CACHE[T] = build_nc(T)
    return _NC_CACHE[T]


def run(inputs, T=T_FULL, **kw):
    nc = _get_nc(T)
    in_maps = prep_in_maps(inputs, T)
    res = run_bass_kernel_spmd(nc, in_maps, core_ids=list(range(N_CORES)), **kw)
    return assemble(res.results, T), res


def kernel(**inputs):
    out, _ = run(inputs)
    return out


# revision 26
# speedup vs baseline: 1.0433x; 1.0433x over previous
"""Causal self-attention (GQA, head-indexed RoPE) on 8 Trainium2 NeuronCores.

Sharding: 2-way data parallel over batch x 4-way tensor parallel over GQA
groups (4 q heads + 1 kv head per core). RoPE in this module is indexed by
the *head* axis (not sequence), so it is a fixed per-head linear map on the
head dim -- folded into the q/k projection weights on the host, along with
the 1/sqrt(d) scale (into q) and the v bias (into the c_proj bias).

All matmuls run in bf16 (same 1 row/cycle PE stream as fp32r, but weight
loads hit the fast-weight-load path and every byte count halves; measured
end-to-end rel err ~4e-3 vs the 2e-2 gate).

Device program per core (identical SPMD program, inputs differ):
  1. qT/kT/vT = W.T-slices @ xT (PSUM accum over C chunks, s-major so PSUM
     banks rotate without stalling the PE), +bias via ACT, v transposed to
     [t, d] layout with PE transposes.
  2. per head, per 512-wide q chunk, per 128-wide k block: S^T tile = K Q^T,
     exp on ACT, causal handled by restricting the diagonal blocks' column
     range (plus one 128x128 triangle multiply on DVE); P V and the ones-
     matmul row sums accumulate in PSUM with a 2-block software pipeline so
     the PE never waits on the ACT exp. DVE reciprocal + multiply normalize
     into yT. After each head, a 4-rank (batch-subgroup) AllToAll reshards
     that head's yT from head-sharded to t-sharded.
  3. c_proj is computed incrementally: as each head-shard lands, its 4
     contraction chunks are matmul'd (PSUM, rotating through the attention
     st pool's banks) and DVE-accumulated into SBUF f32 accumulators that
     were initialized with the output bias. Only the last head-shard's
     round remains after attention ends.
"""

import numpy as np
import ml_dtypes

import concourse.mybir as mybir
import concourse.tile as tile
from concourse import bacc
from concourse.bass_utils import run_bass_kernel_spmd

F32 = mybir.dt.float32
BF16 = mybir.dt.bfloat16
NP_BF16 = ml_dtypes.bfloat16

# Model constants (hardcoded per contract)
B, T_FULL, C = 2, 2048, 2048
NH, G = 16, 4
NKV = NH // G          # 4 kv heads = 4 groups
HD = C // NH           # 128
BASE = 10000.0
N_CORES = 8
HPC = NH // NKV        # 4 q heads per core


# --------------------------------------------------------------------------
# device program
# --------------------------------------------------------------------------

def build_nc(T=T_FULL):
    assert T % 512 == 0
    TC = T // 512          # 512-wide q chunks in attention
    KB = T // 128          # 128-wide k blocks
    CCH = C // 128         # contraction chunks over C
    TS = T // NKV          # c_proj t-shard rows per core
    NTL = TS // 128        # 128-row t tiles in this core's shard

    nc = bacc.Bacc(
        "TRN2",
        target_bir_lowering=False,
        debug=False,
        enable_asserts=False,
        num_devices=N_CORES,
    )

    xT_d = nc.dram_tensor("xT", [128, TC, CCH, 512], BF16, kind="ExternalInput").ap()
    wT_d = nc.dram_tensor("wT", [128, CCH, 6 * HD], BF16, kind="ExternalInput").ap()
    qkb_d = nc.dram_tensor("qkb", [HD, 5], F32, kind="ExternalInput").ap()
    tri_d = nc.dram_tensor("tri", [128, 128], BF16, kind="ExternalInput").ap()
    ones_d = nc.dram_tensor("ones", [128, 128], BF16, kind="ExternalInput").ap()
    ident_d = nc.dram_tensor("ident", [128, 128], BF16, kind="ExternalInput").ap()
    sel_d = nc.dram_tensor("sel", [128, 2], F32, kind="ExternalInput").ap()
    cpT_d = nc.dram_tensor("cpT", [128, HPC, NKV, C], BF16, kind="ExternalInput").ap()
    cpb_d = nc.dram_tensor("cpb", [128, C], F32, kind="ExternalInput").ap()
    out_d = nc.dram_tensor("out", [TS, C], F32, kind="ExternalOutput").ap()

    Act = mybir.ActivationFunctionType
    Alu = mybir.AluOpType
    RG = [list(range(N_CORES))]

    with tile.TileContext(nc) as tc:
        # ---------- DRAM singles for the per-head AllToAll (content
        # duplicated to both batch groups to stay SPMD-symmetric; the
        # receiver selects its batch half with a per-core 0/1 mask) ----------
        warm_in, free_wi = tc.tile([N_CORES * 128, 4], BF16, space="DRAM",
                                   name="a2a_warm_in")
        warm_out, free_wo = tc.tile([N_CORES * 128, 4], BF16, space="DRAM",
                                    name="a2a_warm_out")
        a2a_in, a2a_out, a2a_free = [], [], []
        a2a_free.append(free_wi)
        a2a_free.append(free_wo)
        for h in range(HPC):
            ti, fi = tc.tile([N_CORES * 128, TS], BF16, space="DRAM",
                             name=f"a2a_in{h}")
            a2a_in.append(ti)
            a2a_free.append(fi)
        for h in range(HPC):
            to, fo = tc.tile([N_CORES * 128, TS], BF16, space="DRAM",
                             name=f"a2a_out{h}")
            a2a_out.append(to)
            a2a_free.append(fo)


        # ---------- persistent SBUF ----------
        frees = []

        def ptile(shape, dt, name):
            t, f = tc.tile(shape, dt, name=name)
            frees.append(f)
            return t

        out_acc = ptile([128, 16, 512], F32, "out_acc")
        cpb_sb = ptile([128, C], F32, "cpb_sb")
        qk_sb = ptile([128, 5, T], BF16, "qk_sb")     # 4 qT + kT, d-major
        v_sb = ptile([128, KB, 128], BF16, "v_sb")    # v in [t, d] layout
        yT_sb = ptile([128, HPC, T], BF16, "yT_sb")
        ybig = [ptile([128, CCH, 128], BF16, f"ybig{tl}") for tl in range(NTL)]
        ones_sb = ptile([128, 128], BF16, "ones_sb")
        ident_sb = ptile([128, 128], BF16, "ident_sb")
        tri_sb = ptile([128, 128], BF16, "tri_sb")
        qkb_sb = ptile([128, 5], F32, "qkb_sb")
        sel_sb = ptile([128, 2], F32, "sel_sb")

        nc.sync.dma_start(sel_sb[:], sel_d)
        nc.sync.dma_start(cpb_sb[:], cpb_d)
        nc.sync.dma_start(ones_sb[:], ones_d)
        nc.sync.dma_start(ident_sb[:], ident_d)
        nc.sync.dma_start(tri_sb[:], tri_d)
        nc.sync.dma_start(qkb_sb[:], qkb_d)

        # tiny throwaway AllToAll: absorbs NRT's first-collective trigger
        # latency + ring warmup during phase 1 (payload content irrelevant)
        nc.gpsimd.collective_compute(
            "AllToAll",
            mybir.AluOpType.bypass,
            replica_groups=[list(range(N_CORES))],
            ins=[warm_in.opt()],
            outs=[warm_out.opt()],
        )

        # ---------- phase 1: qkv projections ----------
        with (
            tc.tile_pool(name="w_pool", bufs=1) as w_pool,
            tc.tile_pool(name="x_pool", bufs=3) as x_pool,
            tc.tile_pool(name="qkv_ps", bufs=1, space="PSUM") as qkv_ps,
            tc.tile_pool(name="tp_ps", bufs=2, space="PSUM") as tp_ps,
            tc.tile_pool(name="vtmp_pool", bufs=2) as vtmp_pool,
        ):
            # per-cc-slice DMAs so the first matmuls only wait on their own
            # 1KB/partition slice, not the whole 3MB/2MB transfer
            w_sb = w_pool.tile([128, CCH, 6 * HD], BF16, name="w_sb")

            def drain_s(s, t, ps):
                if s < 5:
                    nc.scalar.activation(
                        qk_sb[:, s, t * 512:(t + 1) * 512],
                        ps[s][:],
                        Act.Identity,
                        bias=qkb_sb[:, s:s + 1],
                        scale=1.0,
                    )
                else:
                    vtmp = vtmp_pool.tile([128, 512], BF16, name="vtmp")
                    nc.any.tensor_copy(out=vtmp[:], in_=ps[5][:])
                    for i in range(4):
                        tp = tp_ps.tile([128, 128], BF16, name="tp")
                        nc.tensor.transpose(
                            tp[:], vtmp[:, i * 128:(i + 1) * 128], ident_sb[:]
                        )
                        nc.any.tensor_copy(out=v_sb[:, t * 4 + i, :], in_=tp[:])

            for t in range(TC):
                xt = x_pool.tile([128, CCH, 512], BF16, name="xt")
                for cc in range(CCH):
                    if t == 0:
                        nc.sync.dma_start(w_sb[:, cc, :], wT_d[:, cc, :])
                    nc.sync.dma_start(xt[:, cc, :], xT_d[:, t, cc, :])
                ps = [qkv_ps.tile([128, 512], F32, name=f"qkv{s}") for s in range(6)]
                if t == 0:
                    # cc-major: consumes one (w, x) slice pair per 6 matmuls,
                    # matching the DMA arrival rate at kernel start
                    for cc in range(CCH):
                        for s in range(6):
                            nc.tensor.matmul(
                                ps[s][:],
                                lhsT=w_sb[:, cc, s * 128:(s + 1) * 128],
                                rhs=xt[:, cc, :],
                                start=(cc == 0),
                                stop=(cc == CCH - 1),
                            )
                    for s in range(6):
                        drain_s(s, t, ps)
                else:
                    # s-major: each PSUM bank finishes early and drains while
                    # the next one computes
                    for s in range(6):
                        for cc in range(CCH):
                            nc.tensor.matmul(
                                ps[s][:],
                                lhsT=w_sb[:, cc, s * 128:(s + 1) * 128],
                                rhs=xt[:, cc, :],
                                start=(cc == 0),
                                stop=(cc == CCH - 1),
                            )
                        drain_s(s, t, ps)

        # ---------- phase 2: attention + per-head AllToAll + streamed c_proj
        with (
            tc.tile_pool(name="st_ps", bufs=3, space="PSUM") as st_ps,
            tc.tile_pool(name="yr_ps", bufs=2, space="PSUM") as yr_ps,
            tc.tile_pool(name="pt_pool", bufs=4) as pt_pool,
            tc.tile_pool(name="nrm_pool", bufs=2) as nrm_pool,
            tc.tile_pool(name="cp_pool", bufs=4) as cp_pool,
            tc.tile_pool(name="ysel_pool", bufs=4) as ysel_pool,
        ):
            cpT_tiles = {}

            def ysel(hs):
                # select this core's batch half of head-shard hs into ybig
                for tl in range(NTL):
                    lo = ysel_pool.tile([128, NKV, 128], BF16, name="lo")
                    hi = ysel_pool.tile([128, NKV, 128], BF16, name="hi")
                    nc.sync.dma_start(
                        lo[:],
                        a2a_out[hs][0:512, tl * 128:(tl + 1) * 128]
                        .rearrange("(g p) t -> p g t", p=128),
                    )
                    nc.sync.dma_start(
                        hi[:],
                        a2a_out[hs][512:1024, tl * 128:(tl + 1) * 128]
                        .rearrange("(g p) t -> p g t", p=128),
                    )
                    tmp = ysel_pool.tile([128, NKV, 128], BF16, name="tmp")
                    nc.vector.tensor_scalar_mul(tmp[:], lo[:], sel_sb[:, 0:1])
                    nc.vector.scalar_tensor_tensor(
                        out=ybig[tl][:, NKV * hs:NKV * (hs + 1), :],
                        in0=hi[:],
                        scalar=sel_sb[:, 1:2],
                        in1=tmp[:],
                        op0=Alu.mult,
                        op1=Alu.add,
                    )

            def fetch_cpT(hs):
                ct = cp_pool.tile([128, NKV, C], BF16, name="cpT")
                nc.sync.dma_start(ct[:], cpT_d[:, hs])
                cpT_tiles[hs] = ct

            def cproj_round(hs):
                # accumulate head-shard hs's 4 contraction chunks into the
                # SBUF f32 accumulators (init with bias on the first round)
                ct = cpT_tiles.pop(hs)
                for tl in range(NTL):
                    for j in range(4):
                        cps = st_ps.tile([128, 512], F32, name="stp")
                        for g in range(NKV):
                            nc.tensor.matmul(
                                cps[:],
                                lhsT=ybig[tl][:, NKV * hs + g, :],
                                rhs=ct[:, g, j * 512:(j + 1) * 512],
                                start=(g == 0),
                                stop=(g == NKV - 1),
                            )
                        idx = 4 * j + tl
                        prev = (cpb_sb[:, j * 512:(j + 1) * 512] if hs == 0
                                else out_acc[:, idx, :])
                        nc.vector.scalar_tensor_tensor(
                            out=out_acc[:, idx, :],
                            in0=cps[:],
                            scalar=1.0,
                            in1=prev,
                            op0=Alu.mult,
                            op1=Alu.add,
                        )
                        if hs == HPC - 1:
                            nc.sync.dma_start(
                                out_d[tl * 128:(tl + 1) * 128,
                                      j * 512:(j + 1) * 512],
                                out_acc[:, idx, :],
                            )

            for hs in range(HPC):
                fetch_cpT(hs)

            for h in range(HPC):
                for qc in range(TC):
                    kb_max = 4 * qc + 4
                    yps = yr_ps.tile([128, 512], F32, name="yps")
                    rps = yr_ps.tile([128, 512], F32, name="rps")
                    pend = []

                    def flush_one():
                        kb0, pt0, c00 = pend.pop(0)
                        nc.tensor.matmul(
                            yps[:, c00:512],
                            lhsT=v_sb[:, kb0, :],
                            rhs=pt0[:, c00:512],
                            start=(kb0 == 0),
                            stop=(kb0 == kb_max - 1),
                        )
                        nc.tensor.matmul(
                            rps[:, c00:512],
                            lhsT=ones_sb[:],
                            rhs=pt0[:, c00:512],
                            start=(kb0 == 0),
                            stop=(kb0 == kb_max - 1),
                        )

                    for kb in range(kb_max):
                        m = kb - 4 * qc
                        c0 = 128 * m if m >= 0 else 0
                        stp = st_ps.tile([128, 512], F32, name="stp")
                        nc.tensor.matmul(
                            stp[:, c0:512],
                            lhsT=qk_sb[:, 4, kb * 128:(kb + 1) * 128],
                            rhs=qk_sb[:, h, qc * 512 + c0:(qc + 1) * 512],
                            start=True,
                            stop=True,
                        )
                        pt = pt_pool.tile([128, 512], BF16, name="pt")
                        nc.scalar.activation(pt[:, c0:512], stp[:, c0:512], Act.Exp)
                        if m >= 0:
                            nc.vector.tensor_mul(
                                out=pt[:, c0:c0 + 128],
                                in0=pt[:, c0:c0 + 128],
                                in1=tri_sb[:],
                            )
                        pend.append((kb, pt, c0))
                        if len(pend) > 2:
                            flush_one()
                    while pend:
                        flush_one()

                    # softmax normalize on DVE (ACT Ln+Exp would be cheaper
                    # per element but walrus puts Ln and Exp in different
                    # table sets -> 2.7us ACT_TABLE_LOAD per alternation)
                    recip = nrm_pool.tile([128, 512], F32, name="recip")
                    nc.vector.reciprocal(recip[:], rps[:])
                    nc.vector.tensor_mul(
                        out=yT_sb[:, h, qc * 512:(qc + 1) * 512],
                        in0=yps[:],
                        in1=recip[:],
                    )
                    # this qc's columns are exactly a2a chunk gd == qc:
                    # ship them now (identical content to both batch groups'
                    # rank so the program stays core-independent)
                    for bd in range(2):
                        j = 4 * bd + qc
                        nc.sync.dma_start(
                            a2a_in[h][128 * j:128 * (j + 1), :],
                            yT_sb[:, h, qc * TS:(qc + 1) * TS],
                        )
                    # head h-2's select runs mid-head on otherwise-idle DVE
                    # slots; its AllToAll landed a full head ago
                    if qc == 1 and h >= 2:
                        ysel(h - 2)

                nc.gpsimd.collective_compute(
                    "AllToAll",
                    Alu.bypass,
                    replica_groups=RG,
                    ins=[a2a_in[h].opt()],
                    outs=[a2a_out[h].opt()],
                )
                if h >= 2:
                    cproj_round(h - 2)

            ysel(HPC - 2)
            cproj_round(HPC - 2)
            ysel(HPC - 1)
            cproj_round(HPC - 1)

        for f in reversed(frees):
            f()
        for f in reversed(a2a_free):
            f()

    nc.compile()
    return nc


# --------------------------------------------------------------------------
# host-side preparation
# --------------------------------------------------------------------------

def _rot_mat(angle_idx):
    half = HD // 2
    theta = 1.0 / (BASE ** (np.arange(half, dtype=np.float64) * 2.0 / HD))
    ang = angle_idx * theta
    c, s = np.cos(ang), np.sin(ang)
    R = np.zeros((HD, HD), dtype=np.float64)
    idx = np.arange(half)
    R[2 * idx, 2 * idx] = c
    R[2 * idx, 2 * idx + 1] = -s
    R[2 * idx + 1, 2 * idx] = s
    R[2 * idx + 1, 2 * idx + 1] = c
    return R


def prep_in_maps(inputs, T=T_FULL):
    TC = T // 512
    CCH = C // 128
    x = np.asarray(inputs["x"], np.float32)
    wq_w = np.asarray(inputs["wq_w"], np.float64)
    wq_b = np.asarray(inputs["wq_b"], np.float64)
    wk_w = np.asarray(inputs["wk_w"], np.float64)
    wk_b = np.asarray(inputs["wk_b"], np.float64)
    wv_w = np.asarray(inputs["wv_w"], np.float64)
    wv_b = np.asarray(inputs["wv_b"], np.float64)
    cp_w = np.asarray(inputs["cp_w"], np.float64)
    cp_b = np.asarray(inputs["cp_b"], np.float64)

    scale = 1.0 / np.sqrt(HD)
    wq_eff = np.empty((C, C), np.float64)
    bq_eff = np.empty(C, np.float64)
    for h in range(NH):
        R = _rot_mat(h)
        sl = slice(HD * h, HD * (h + 1))
        wq_eff[sl] = R @ wq_w[sl] * scale
        bq_eff[sl] = R @ wq_b[sl] * scale
    wk_eff = np.empty((C // G, C), np.float64)
    bk_eff = np.empty(C // G, np.float64)
    for g in range(NKV):
        R = _rot_mat(g)
        sl = slice(HD * g, HD * (g + 1))
        wk_eff[sl] = R @ wk_w[sl]
        bk_eff[sl] = R @ wk_b[sl]

    bv_exp = np.zeros(C, np.float64)
    for h in range(NH):
        g = h // G
        bv_exp[HD * h:HD * (h + 1)] = wv_b[HD * g:HD * (g + 1)]
    cp_b_eff = (cp_b + cp_w @ bv_exp).astype(np.float32)

    i = np.arange(128)
    tri = (i[None, :] >= i[:, None]).astype(NP_BF16)       # [kpos, qpos]
    ones = np.ones((128, 128), NP_BF16)
    ident = np.eye(128, dtype=NP_BF16)

    # c_proj weights packed [ci, hs, g, c] so the per-head-shard tile is a
    # contiguous DMA: row (hs, g, ci) <- cp column for global head (4g + hs)
    cpT_pack = np.empty((128, HPC, NKV, C), NP_BF16)
    for hs in range(HPC):
        for g in range(NKV):
            H = NKV * g + hs
            cpT_pack[:, hs, g, :] = \
                cp_w[:, 128 * H:128 * (H + 1)].astype(np.float32).T.astype(NP_BF16)
    cpb_full = np.tile(cp_b_eff[None, :], (128, 1))

    # x packed [ci, t_chunk, co, f] so each t-chunk is one contiguous DMA
    xT = []
    for b in range(B):
        arr = np.ascontiguousarray(x[b, :T].T).astype(NP_BF16)   # [C, T]
        xT.append(np.ascontiguousarray(
            arr.reshape(CCH, 128, TC, 512).transpose(1, 2, 0, 3)))

    in_maps = []
    for core in range(N_CORES):
        b, g = divmod(core, NKV)
        wq_g = wq_eff[512 * g:512 * (g + 1)]          # [512, C]
        wk_g = wk_eff[128 * g:128 * (g + 1)]          # [128, C]
        wv_g = wv_w[128 * g:128 * (g + 1)]            # [128, C]
        wT = np.concatenate(
            [wq_g.T, wk_g.T, wv_g.T], axis=1).astype(np.float32)  # [C, 768]
        wT_pack = np.ascontiguousarray(
            wT.reshape(CCH, 128, 6 * HD).transpose(1, 0, 2)).astype(NP_BF16)
        qkb = np.concatenate(
            [bq_eff[512 * g:512 * (g + 1)].astype(np.float32).reshape(4, 128).T,
             bk_eff[128 * g:128 * (g + 1)].astype(np.float32).reshape(1, 128).T],
            axis=1,
        )                                              # [128, 5]
        sel = np.zeros((128, 2), np.float32)
        sel[:, b] = 1.0
        in_maps.append({
            "xT": xT[b],
            "wT": wT_pack,
            "qkb": np.ascontiguousarray(qkb),
            "tri": tri,
            "ones": ones,
            "ident": ident,
            "sel": sel,
            "cpT": cpT_pack,
            "cpb": cpb_full,
        })
    return in_maps


def assemble(results, T=T_FULL):
    TS = T // NKV
    out = np.empty((B, T, C), np.float32)
    for core in range(N_CORES):
        b, g = divmod(core, NKV)
        out[b, TS * g:TS * (g + 1), :] = results[core]["out"]
    return out


_NC_CACHE = {}


def _get_nc(T=T_FULL):
    if T not in _NC_CACHE:
        _NC_CACHE[T] = build_nc(T)
    return _NC_CACHE[T]


def run(inputs, T=T_FULL, **kw):
    nc = _get_nc(T)
    in_maps = prep_in_maps(inputs, T)
    res = run_bass_kernel_spmd(nc, in_maps, core_ids=list(range(N_CORES)), **kw)
    return assemble(res.results, T), res


def kernel(**inputs):
    out, _ = run(inputs)
    return out
